# revision 1
# baseline (speedup 1.0000x reference)
"""MixerDiffAttention Trainium2 kernel.

Sharding: 8 cores = 8 head-pairs (tensor parallel over head-pair dim).
Each core processes BOTH batches for its head-pair: the per-core weight
slice (768 qkv cols + 256 gate cols) stays SBUF-resident, and each core
produces the disjoint output slice y[:, :, hp*256:(hp+1)*256].

Per core, per batch:
  Phase 1 (per 128-token tile): PE-transpose x -> xT; qkv+gate matmul
    (fp32r, K=2048 accumulated); q/k RMSNorm stats via ACT Square+accum,
    rstd = Exp(-0.5*Ln(ms+eps)); RoPE on DVE; PE-transpose rope'd q/k to
    feature-major [hd, T]; v -> SBUF with an appended ones column (gives
    softmax row-sums for free); gate raw -> SBUF (SiLU batched later).
  Phase 2 (per 256-query chunk): transposed scores scT[Tk,Tq] =
    kTile.T @ qChunk (fp32r); no max-subtraction (q,k are RMSNorm'd so
    |scores| <= sqrt(128); exp is safe); causal diag masks added on DVE;
    probs = Exp(scale*scT) on ACT -> fp32r; av accumulates
    probsT.T @ [v|1] in PSUM; epilogue fuses diff-attention combine,
    SiLU gate, group RMSNorm (rsqrt via Exp(-0.5*Ln(x) + ln(c))).
"""
import sys
sys.path.insert(0, "/opt/trn_rl_repo")
import numpy as np
import concourse.bass as bass
from concourse import bacc
import concourse.tile as tile
from concourse import mybir
from concourse.bass_utils import run_bass_kernel_spmd

F32 = mybir.dt.float32
F32R = mybir.dt.float32r
AF = mybir.ActivationFunctionType
ALU = mybir.AluOpType

B, T, D, HD = 2, 2048, 2048, 128
KT = D // 128          # 16 contraction tiles
TT = T // 128          # 16 token tiles
CH = 256               # query-chunk width in phase 2
NCH = T // CH          # 8 chunks
N_CORES = 8
LAMBDA_INIT = 0.8 - 0.6 * float(np.exp(-0.3 * 6))
ONE_MINUS_LI = 1.0 - LAMBDA_INIT
SCALE = float(HD ** -0.5)
EPS = 1e-6
NEG = -1e9


def _bcast_mid(ap, n):
    # [P, F] AP -> [P, n, F] with a zero-stride middle dim
    return bass.AP(tensor=ap.tensor, offset=ap.offset,
                   ap=[ap.ap[0], [0, n], *ap.ap[1:]])


def _rsqrt_dve(nc, pool, ss_ap, width, mean_div, tag):
    """rstd = (ss/mean_div + EPS) ** -0.5 entirely on DVE.

    Quake-III bit-trick seed + 2 Newton iterations (~5e-6 rel err); avoids
    ACT Ln/Sqrt so the whole kernel stays inside one ACT table set."""
    I32 = mybir.dt.int32
    ms = pool.tile([128, width], F32, name=tag + "_ms")
    nc.vector.tensor_scalar(out=ms[:], in0=ss_ap, scalar1=1.0 / mean_div,
                            scalar2=EPS, op0=ALU.mult, op1=ALU.add)
    iv = pool.tile([128, width], I32, name=tag + "_iv")
    nc.vector.tensor_scalar(out=iv[:], in0=ms[:].bitcast(I32), scalar1=1,
                            scalar2=None, op0=ALU.logical_shift_right)
    y = pool.tile([128, width], F32, name=tag + "_y")
    nc.vector.tensor_scalar(out=y[:].bitcast(I32), in0=iv[:], scalar1=-1,
                            scalar2=0x5F3759DF, op0=ALU.mult, op1=ALU.add)
    a = pool.tile([128, width], F32, name=tag + "_a")
    u = pool.tile([128, width], F32, name=tag + "_u")
    for _ in range(2):
        nc.vector.tensor_mul(a[:], y[:], y[:])
        nc.vector.tensor_mul(a[:], a[:], ms[:])
        nc.vector.tensor_scalar(out=u[:], in0=a[:], scalar1=-0.5, scalar2=1.5,
                                op0=ALU.mult, op1=ALU.add)
        nc.vector.tensor_mul(y[:], y[:], u[:])
    return y


def build(tt=TT, nb=B, phases=2):
    nch = tt * 128 // CH
    nc = bacc.Bacc("TRN2", target_bir_lowering=False, debug=False,
                   num_devices=N_CORES)
    xt_d = nc.dram_tensor("xt", [nb, D, tt * 128], F32R, kind="ExternalInput").ap()
    w_d = nc.dram_tensor("wcat", [D, 1024], F32R, kind="ExternalInput").ap()
    cos_d = nc.dram_tensor("cos", [tt * 128, 64], F32, kind="ExternalInput").ap()
    sin_d = nc.dram_tensor("sin", [tt * 128, 64], F32, kind="ExternalInput").ap()
    mask_d = nc.dram_tensor("masks", [128, 2, CH], F32, kind="ExternalInput").ap()
    id_d = nc.dram_tensor("ident", [128, 128], F32, kind="ExternalInput").ap()
    ones_d = nc.dram_tensor("ones", [128, 4], F32R, kind="ExternalInput").ap()
    y_d = nc.dram_tensor("y", [nb, tt * 128, 256], F32, kind="ExternalOutput").ap()

    with tile.TileContext(nc) as tc:
        with tc.tile_pool(name="bigs", bufs=1) as bigs, \
             tc.tile_pool(name="consts", bufs=1) as consts, \
             tc.tile_pool(name="xtp", bufs=4) as xtp:
            # ---- constants / weights ----
            cos_sb = consts.tile([128, tt, 64], F32)
            nc.sync.dma_start(cos_sb[:], cos_d.rearrange("(t p) f -> p t f", p=128))
            sin_sb = consts.tile([128, tt, 64], F32)
            nc.sync.dma_start(sin_sb[:], sin_d.rearrange("(t p) f -> p t f", p=128))
            mask_sb = consts.tile([128, 2, CH], F32)
            nc.sync.dma_start(mask_sb[:], mask_d)
            id_sb = consts.tile([128, 128], F32)
            nc.sync.dma_start(id_sb[:], id_d)
            ones_sb = consts.tile([128, 4], F32R)
            nc.sync.dma_start(ones_sb[:], ones_d)

            # prefetch the first token tiles ahead of the big wcat load so
            # the PE can start within a few us (DMA queues drain in order)
            xT_pre = {}
            xt_v0 = xt_d[0].rearrange("(k p) t -> p k t", p=128)
            for t in range(min(3, tt)):
                xp = xtp.tile([128, KT, 128], F32R, name="xT_t")
                for kh in range(2):
                    nc.sync.dma_start(
                        xp[:, kh * 8:(kh + 1) * 8, :],
                        xt_v0[:, kh * 8:(kh + 1) * 8, t * 128:(t + 1) * 128])
                xT_pre[t] = xp
            wcat = bigs.tile([128, KT, 1024], F32R)
            w_v = w_d.rearrange("(k p) c -> p k c", p=128)
            for k in range(KT):
                nc.sync.dma_start(wcat[:, k, :], w_v[:, k, :])

            # ---- per-batch persistent (reused sequentially) ----
            qkT = bigs.tile([128, 4, tt * 128], F32R)   # q1,q2,k1,k2 feature-major
            v_sb = bigs.tile([128, tt, 260], F32R)      # [tok, v(256)|1|0 pad]
            g_sb = bigs.tile([128, tt, 256], F32)       # gate (raw -> silu'd)

            for b in range(nb):
                xt_v = xt_d[b].rearrange("(k p) t -> p k t", p=128)
                # ================= Phase 1 =================
                with tc.tile_pool(name="p1s", bufs=2) as p1s, \
                     tc.tile_pool(name="p1t", bufs=3) as p1t, \
                     tc.tile_pool(name="tp_ps", bufs=2, space="PSUM") as tp_ps, \
                     tc.tile_pool(name="mm_ps", bufs=3, space="PSUM") as mm_ps:
                    for t in range(tt):
                        if b == 0 and t in xT_pre:
                            xT_t = xT_pre.pop(t)
                        else:
                            xT_t = xtp.tile([128, KT, 128], F32R, name="xT_t")
                            for kh in range(2):
                                nc.sync.dma_start(
                                    xT_t[:, kh * 8:(kh + 1) * 8, :],
                                    xt_v[:, kh * 8:(kh + 1) * 8,
                                         t * 128:(t + 1) * 128])
                        qk_ps = mm_ps.tile([128, 512], F32, name="qk_ps")
                        vg_ps = mm_ps.tile([128, 512], F32, name="vg_ps")
                        for k in range(KT):
                            nc.tensor.matmul(qk_ps[:], xT_t[:, k, :], wcat[:, k, 0:512],
                                             start=(k == 0), stop=(k == KT - 1))
                            nc.tensor.matmul(vg_ps[:], xT_t[:, k, :], wcat[:, k, 512:1024],
                                             start=(k == 0), stop=(k == KT - 1))
                        # ---- q/k rmsnorm stats (ACT) ----
                        ss = p1t.tile([128, 4], F32, name="ss")
                        sq_scr = p1t.tile([128, 128], F32, name="sq_scr")
                        for h in range(4):
                            nc.scalar.activation(sq_scr[:], qk_ps[:, h * 128:(h + 1) * 128],
                                                 AF.Square, accum_out=ss[:, h:h + 1])
                        rstd = _rsqrt_dve(nc, p1t, ss[:], 4, HD, "rq")
                        # ---- rope (DVE, batched over the 4 head-cols) ----
                        qk_v = qk_ps[:].rearrange("p (h d) -> p h d", h=4)
                        h1, h2 = qk_v[:, :, 0:64], qk_v[:, :, 64:128]
                        cos_b = _bcast_mid(cos_sb[:, t, :], 4)
                        sin_b = _bcast_mid(sin_sb[:, t, :], 4)
                        ra = p1t.tile([128, 4, 64], F32, name="ra")
                        rb = p1t.tile([128, 4, 64], F32, name="rb")
                        rot = p1t.tile([128, 4, 128], F32, name="rot")
                        nc.vector.tensor_mul(ra[:], h1, cos_b)
                        nc.vector.tensor_mul(rb[:], h2, sin_b)
                        nc.vector.tensor_add(rot[:, :, 0:64], ra[:], rb[:])
                        nc.vector.tensor_mul(ra[:], h2, cos_b)
                        nc.vector.tensor_mul(rb[:], h1, sin_b)
                        nc.vector.tensor_sub(rot[:, :, 64:128], ra[:], rb[:])
                        qrot = p1t.tile([128, 4, 128], F32, name="qrot")
                        for h in range(4):
                            nc.vector.tensor_scalar_mul(qrot[:, h, :], in0=rot[:, h, :],
                                                        scalar1=rstd[:, h:h + 1])
                        # ---- q/k transposes: 4 blocks -> one PSUM bank,
                        #      single batched copyback to feature-major qkT ----
                        tq = tp_ps.tile([128, 512], F32, name="tq")
                        for h in range(4):
                            nc.tensor.transpose(tq[:, h * 128:(h + 1) * 128],
                                                qrot[:, h, :], id_sb[:])
                        nc.scalar.copy(qkT[:, :, t * 128:(t + 1) * 128],
                                       tq[:].rearrange("p (h d) -> p h d", h=4))
                        # ---- v (+ones) and raw gate (DVE) ----
                        nc.vector.tensor_copy(v_sb[:, t, 0:256], vg_ps[:, 0:256])
                        nc.vector.tensor_copy(v_sb[:, t, 256:260], ones_sb[:])
                        nc.vector.tensor_copy(g_sb[:, t, :], vg_ps[:, 256:512])
                if phases < 2:
                    for t in range(tt):
                        th = p1t.tile([128, 256], F32, name="th")
                        nc.scalar.activation(th[:], g_sb[:, t, :], AF.Tanh,
                                             scale=0.5)
                        nc.vector.scalar_tensor_tensor(
                            g_sb[:, t, :], th[:], 1.0, g_sb[:, t, :],
                            op0=ALU.add, op1=ALU.mult)
                    with tc.tile_pool(name="dump", bufs=2) as dump:
                        for t in range(tt):
                            d_t = dump.tile([128, 256], F32, name="d_t")
                            nc.vector.tensor_copy(d_t[:], v_sb[:, t, 0:256])
                            nc.vector.tensor_add(d_t[:], d_t[:], g_sb[:, t, :])
                            nc.sync.dma_start(
                                y_d[b, t * 128:(t + 1) * 128, :], d_t[:])
                    continue
                # ================= Phase 2 =================
                with tc.tile_pool(name="p2s", bufs=4) as p2s, \
                     tc.tile_pool(name="p2e", bufs=3) as p2e, \
                     tc.tile_pool(name="sc_ps", bufs=2, space="PSUM") as sc_ps, \
                     tc.tile_pool(name="av_ps", bufs=6, space="PSUM") as av_ps:
                    # SiLU via tanh (stays in the exp_and_others ACT set):
                    # silu(x) = x*(tanh(x/2)+1)/2; the 1/2 is restored at the
                    # epilogue gate-multiply. Runs here so it overlaps
                    # attention instead of serializing the phase boundary.
                    for t in range(tt):
                        th = p2e.tile([128, 256], F32, name="th")
                        nc.scalar.activation(th[:], g_sb[:, t, :], AF.Tanh,
                                             scale=0.5)
                        nc.vector.scalar_tensor_tensor(
                            g_sb[:, t, :], th[:], 1.0, g_sb[:, t, :],
                            op0=ALU.add, op1=ALU.mult)
                    for c in range(nch):
                        yps = {}
                        for var in range(2):
                            for m in range(2):
                                yps[(var, m)] = av_ps.tile([128, 260], F32, name="yacc")
                            qch = qkT[:, var, c * CH:(c + 1) * CH]
                            for jp in range(c + 1):
                                j0 = 2 * jp
                                scp = sc_ps.tile([128, 2, CH], F32, name="sc")
                                for jj in range(2):
                                    nc.tensor.matmul(
                                        scp[:, jj, :],
                                        qkT[:, 2 + var, (j0 + jj) * 128:(j0 + jj + 1) * 128],
                                        qch, start=True, stop=True)
                                if jp == c:   # diagonal pair -> causal masks
                                    msc = p2s.tile([128, 2, CH], F32, name="msc")
                                    for jj in range(2):
                                        nc.vector.scalar_tensor_tensor(
                                            msc[:, jj, :], scp[:, jj, :], 1.0,
                                            mask_sb[:, jj, :],
                                            op0=ALU.mult, op1=ALU.add)
                                    exp_src = msc
                                else:
                                    exp_src = scp
                                probs = p2s.tile([128, 2, CH], F32R, name="probs")
                                nc.scalar.activation(probs[:], exp_src[:], AF.Exp,
                                                     scale=SCALE)
                                for jj in range(2):
                                    j = j0 + jj
                                    for m in range(2):
                                        nc.tensor.matmul(
                                            yps[(var, m)][:],
                                            probs[:, jj, m * 128:(m + 1) * 128],
                                            v_sb[:, j, :],
                                            start=(j == 0), stop=(j == 2 * c + 1))
                        # ---- epilogue ----
                        ssy = p2e.tile([128, 2], F32, name="ssy")
                        ygs = []
                        for m in range(2):
                            y1p, y2p = yps[(0, m)], yps[(1, m)]
                            # v col 256 = 1 -> s1; col 257 = -1/lam -> r2n is
                            # one reciprocal away (no separate -lam multiply)
                            r1 = p2e.tile([128, 1], F32, name="r1")
                            r2n = p2e.tile([128, 1], F32, name="r2n")
                            nc.vector.reciprocal(r1[:], y1p[:, 256:257])
                            nc.vector.reciprocal(r2n[:], y2p[:, 257:258])
                            t1 = p2e.tile([128, 256], F32, name="t1")
                            nc.vector.tensor_scalar_mul(t1[:], in0=y1p[:, 0:256],
                                                        scalar1=r1[:])
                            yt = p2e.tile([128, 256], F32, name="yt")
                            nc.vector.scalar_tensor_tensor(
                                yt[:], y2p[:, 0:256], r2n[:], t1[:],
                                op0=ALU.mult, op1=ALU.add)
                            yg = p2e.tile([128, 256], F32, name="yg", bufs=2)
                            # 0.5 restores silu scale (g_sb holds 2*silu)
                            nc.vector.scalar_tensor_tensor(
                                yg[:], yt[:], 0.5, g_sb[:, 2 * c + m, :],
                                op0=ALU.mult, op1=ALU.mult)
                            sq2 = p2e.tile([128, 256], F32, name="sq2")
                            nc.scalar.activation(sq2[:], yg[:], AF.Square,
                                                 accum_out=ssy[:, m:m + 1])
                            ygs.append(yg)
                        rsy = _rsqrt_dve(nc, p2e, ssy[:], 2, 256, "ry")
                        for m in range(2):
                            qt = 2 * c + m
                            out_t = p2e.tile([128, 256], F32, name="out_t")
                            nc.vector.tensor_scalar(
                                out=out_t[:], in0=ygs[m][:],
                                scalar1=rsy[:, m:m + 1], scalar2=ONE_MINUS_LI,
                                op0=ALU.mult, op1=ALU.mult)
                            nc.sync.dma_start(
                                y_d[b, qt * 128:(qt + 1) * 128, :], out_t[:])
    nc.compile()
    return nc


_NC = None


def prep_in_maps(hidden_states, W_qkv, lambda_q1, lambda_k1, lambda_q2,
                 lambda_k2, W_g):
    x = np.asarray(hidden_states, dtype=np.float32)
    xt = np.ascontiguousarray(x.transpose(0, 2, 1))
    W_qkv = np.asarray(W_qkv, dtype=np.float32)
    W_g = np.asarray(W_g, dtype=np.float32)

    t_ar = np.arange(T, dtype=np.float32)
    inv_freq = (1.0 / 10000.0 ** (np.arange(0, HD, 2, dtype=np.float32) / HD)
                ).astype(np.float32)
    freqs = np.outer(t_ar, inv_freq).astype(np.float32)
    cos = np.cos(freqs).astype(np.float32)
    sin = np.sin(freqs).astype(np.float32)

    masks = np.empty((128, 2, CH), dtype=np.float32)
    kk = np.arange(128)[:, None]
    qq = np.arange(CH)[None, :]
    for m in range(2):
        masks[:, m, :] = np.where(m * 128 + kk <= qq, 0.0, NEG)

    ident = np.eye(128, dtype=np.float32)

    lam1 = np.exp(np.sum(np.asarray(lambda_q1, np.float32)
                         * np.asarray(lambda_k1, np.float32), axis=-1))
    lam2 = np.exp(np.sum(np.asarray(lambda_q2, np.float32)
                         * np.asarray(lambda_k2, np.float32), axis=-1))
    lam = (lam1 - lam2 + LAMBDA_INIT).astype(np.float32)   # [8]

    in_maps = []
    for c in range(N_CORES):
        base = 2 * c * 384
        w_cols = [
            W_qkv[:, base:base + 128],            # q1
            W_qkv[:, base + 384:base + 512],      # q2
            W_qkv[:, base + 128:base + 256],      # k1
            W_qkv[:, base + 512:base + 640],      # k2
            W_qkv[:, base + 256:base + 384],      # v1
            W_qkv[:, base + 640:base + 768],      # v2
            W_g[:, c * 256:(c + 1) * 256],        # gate
        ]
        wcat = np.ascontiguousarray(np.concatenate(w_cols, axis=1))
        ones = np.zeros((128, 4), dtype=np.float32)
        ones[:, 0] = 1.0
        ones[:, 1] = -1.0 / lam[c]
        in_maps.append({
            "xt": xt, "wcat": wcat, "cos": cos, "sin": sin,
            "masks": masks, "ident": ident, "ones": ones,
        })

    return in_maps


def kernel(hidden_states, W_qkv, lambda_q1, lambda_k1, lambda_q2, lambda_k2,
           W_g, **run_kwargs):
    global _NC
    if _NC is None:
        _NC = build()
    in_maps = prep_in_maps(hidden_states, W_qkv, lambda_q1, lambda_k1,
                           lambda_q2, lambda_k2, W_g)
    res = run_bass_kernel_spmd(_NC, in_maps, core_ids=list(range(N_CORES)),
                               **run_kwargs)
    out = np.empty((B, T, D), dtype=np.float32)
    for c in range(N_CORES):
        out[:, :, c * 256:(c + 1) * 256] = res.results[c]["y"]
    if run_kwargs:
        return out, res
    return out



# revision 2
# speedup vs baseline: 1.0097x; 1.0097x over previous
"""MixerDiffAttention Trainium2 kernel.

Sharding: 8 cores = 8 head-pairs (tensor parallel). Each core handles both
batches for its head pair; per-core weight slice (768 qkv + 256 gate cols).

Per core the two phases are software-pipelined per batch:
  step s: GEMM tiles 2s, 2s+1  ||  attention chunk s-1 (one pair behind).

Phase 1 (per 128-token tile):
  - qkv+gate GEMM as a 3-term split-fp8 DoubleRow product:
      x = xh + xl, W = Wh + Wl (fp8 e4m3, host-split);
      out = (xh+xl)@Wh + xh@Wl  (lo*lo dropped, ~0.1% error)
    24 DoubleRow matmuls per 512-col half (16x pair(hi_k,lo_k)*Wh_k,
    8x pair(hi_k,hi_k+1)*(Wl_k,Wl_k+1)).
  - qk -> fp16 copy (ACT), sumsq stats via DVE tensor_tensor_reduce,
    rstd via Quake rsqrt (DVE), RoPE in fp16 (DVE 2x), rstd fold (DVE 4x),
    feature-major transpose on PE (fp16), v+gate -> fp16.
Phase 2 (per 256-query chunk, vars = even/odd heads):
  - scores fp16 matmuls -> PSUM f32; diag causal masks (Pool stt);
  - probs = Exp(scale*s - 2) -> fp16 (bias washes in softmax norm);
  - AV fp16 matmuls accumulate [v|1] -> per-query sums come free;
  - epilogue: rho = lam*s1/s2 (reciprocal + 2 tiny ops), yt = rho*y2 - y1,
    gate multiply, group-RMS via TTR + Quake; all row scales (silu 2x, WS,
    s1, sign) wash in the group RMSNorm.
"""
import sys
sys.path.insert(0, "/opt/trn_rl_repo")
import numpy as np
import ml_dtypes
import concourse.bass as bass
from concourse import bacc
import concourse.tile as tile
from concourse import mybir
from concourse.bass_utils import run_bass_kernel_spmd

F32 = mybir.dt.float32
F32R = mybir.dt.float32r
F16 = mybir.dt.float16
F8 = mybir.dt.float8e4
AF = mybir.ActivationFunctionType
ALU = mybir.AluOpType
DR = mybir.MatmulPerfMode.DoubleRow

B, T, D, HD = 2, 2048, 2048, 128
KT = D // 128          # 16 contraction tiles of 128
TT = T // 128          # 16 token tiles
CH = 256               # query-chunk width
NCH = T // CH          # 8 chunks
VW = 257               # v width: 256 + ones column
N_CORES = 8
WS = 32.0              # host weight prescale (washes in rmsnorms)
LAMBDA_INIT = 0.8 - 0.6 * float(np.exp(-0.3 * 6))
ONE_MINUS_LI = 1.0 - LAMBDA_INIT
SCALE = float(HD ** -0.5)
EXP_BIAS = -2.0        # washes in softmax normalization; keeps probs in f16
EPS = 1e-6
NEG = -1e9


def _bcast_mid(ap, n):
    # [P, F] AP -> [P, n, F] with a zero-stride middle dim
    return bass.AP(tensor=ap.tensor, offset=ap.offset,
                   ap=[ap.ap[0], [0, n], *ap.ap[1:]])


def _rsqrt_dve(nc, pool, ss_ap, width, scale1, iters, tag):
    """rstd = (ss*scale1 + EPS) ** -0.5 on DVE (Quake seed + Newton)."""
    I32 = mybir.dt.int32
    ms = pool.tile([128, width], F32, name=tag + "_ms")
    nc.vector.tensor_scalar(out=ms[:], in0=ss_ap, scalar1=scale1,
                            scalar2=EPS, op0=ALU.mult, op1=ALU.add)
    iv = pool.tile([128, width], I32, name=tag + "_iv")
    nc.vector.tensor_scalar(out=iv[:], in0=ms[:].bitcast(I32), scalar1=1,
                            scalar2=None, op0=ALU.logical_shift_right)
    y = pool.tile([128, width], F32, name=tag + "_y")
    nc.vector.tensor_scalar(out=y[:].bitcast(I32), in0=iv[:], scalar1=-1,
                            scalar2=0x5F3759DF, op0=ALU.mult, op1=ALU.add)
    a = pool.tile([128, width], F32, name=tag + "_a")
    u = pool.tile([128, width], F32, name=tag + "_u")
    for _ in range(iters):
        nc.vector.tensor_mul(a[:], y[:], y[:])
        nc.vector.tensor_mul(a[:], a[:], ms[:])
        nc.vector.tensor_scalar(out=u[:], in0=a[:], scalar1=-0.5, scalar2=1.5,
                                op0=ALU.mult, op1=ALU.add)
        nc.vector.tensor_mul(y[:], y[:], u[:])
    return y


def build(nb=B):
    nc = bacc.Bacc("TRN2", target_bir_lowering=False, debug=False,
                   num_devices=N_CORES)
    xh_d = nc.dram_tensor("xh", [nb, D, T], F8, kind="ExternalInput").ap()
    xl_d = nc.dram_tensor("xl", [nb, D, T], F8, kind="ExternalInput").ap()
    wh_d = nc.dram_tensor("wh", [D, 1024], F8, kind="ExternalInput").ap()
    wl_d = nc.dram_tensor("wl", [D, 1024], F8, kind="ExternalInput").ap()
    cos_d = nc.dram_tensor("cos", [T, 64], F16, kind="ExternalInput").ap()
    sin_d = nc.dram_tensor("sin", [T, 64], F16, kind="ExternalInput").ap()
    mask_d = nc.dram_tensor("masks", [128, 384], F32, kind="ExternalInput").ap()
    lam_d = nc.dram_tensor("lam", [128, 1], F32, kind="ExternalInput").ap()
    id_d = nc.dram_tensor("ident", [128, 128], F16, kind="ExternalInput").ap()
    y_d = nc.dram_tensor("y", [nb, T, 256], F32, kind="ExternalOutput").ap()

    TQ = T // 4  # x stripe chunk (512 tokens)

    with tile.TileContext(nc) as tc:
        with tc.tile_pool(name="consts", bufs=1) as consts, \
             tc.tile_pool(name="xtp", bufs=5) as xtp, \
             tc.tile_pool(name="qktp", bufs=2) as qktp, \
             tc.tile_pool(name="vgp", bufs=2) as vgp, \
             tc.tile_pool(name="p1w", bufs=2) as p1w, \
             tc.tile_pool(name="p1s", bufs=8) as p1s, \
             tc.tile_pool(name="p2w", bufs=3) as p2w, \
             tc.tile_pool(name="p2s", bufs=4) as p2s, \
             tc.tile_pool(name="outp", bufs=2) as outp, \
             tc.tile_pool(name="mm_ps", bufs=1, space="PSUM") as mm_ps, \
             tc.tile_pool(name="sc_ps", bufs=2, space="PSUM") as sc_ps, \
             tc.tile_pool(name="yac_ps", bufs=3, space="PSUM") as yac_ps, \
             tc.tile_pool(name="tp_ps", bufs=1, space="PSUM") as tp_ps:

            # ---- constants (w split into k-halves for a fast first tile) ----
            w2 = consts.tile([128, 2, KT, 1024], F8)
            wh_v = wh_d.rearrange("(k p) c -> p k c", p=128)
            wl_v = wl_d.rearrange("(k p) c -> p k c", p=128)
            nc.sync.dma_start(w2[:, 0, 0:8, :], wh_v[:, 0:8, :])
            nc.sync.dma_start(w2[:, 0, 8:16, :], wh_v[:, 8:16, :])
            xq_tiles = {}

            def get_xchunk(b, ch):
                if (b, ch) not in xq_tiles:
                    xq = xtp.tile([128, 2, KT, TQ], F8, name="xq")
                    bsl = slice(ch * TQ, (ch + 1) * TQ)
                    xhv = xh_d[b].rearrange("(k p) t -> p k t", p=128)
                    xlv = xl_d[b].rearrange("(k p) t -> p k t", p=128)
                    nc.sync.dma_start(xq[:, 0, :, :], xhv[:, :, bsl])
                    nc.sync.dma_start(xq[:, 1, :, :], xlv[:, :, bsl])
                    xq_tiles[(b, ch)] = xq
                return xq_tiles[(b, ch)]

            get_xchunk(0, 0)
            nc.sync.dma_start(w2[:, 1, 0:8, :], wl_v[:, 0:8, :])
            nc.sync.dma_start(w2[:, 1, 8:16, :], wl_v[:, 8:16, :])
            cos_sb = consts.tile([128, TT, 64], F16)
            nc.sync.dma_start(cos_sb[:], cos_d.rearrange("(t p) f -> p t f", p=128))
            sin_sb = consts.tile([128, TT, 64], F16)
            nc.sync.dma_start(sin_sb[:], sin_d.rearrange("(t p) f -> p t f", p=128))
            mask_sb = consts.tile([128, 384], F32)
            nc.sync.dma_start(mask_sb[:], mask_d)
            lam_sb = consts.tile([128, 1], F32)   # holds 1/lambda
            nc.sync.dma_start(lam_sb[:], lam_d)
            ebias = consts.tile([128, 1], F32)    # exp bias (washes in norm)
            nc.vector.memset(ebias[:], EXP_BIAS)
            id_sb = consts.tile([128, 128], F16)
            nc.sync.dma_start(id_sb[:], id_d)

            for b in range(nb):
                qkT = qktp.tile([128, TT, 4, 128], F16, name="qkT")
                v16 = vgp.tile([128, KT, VW], F16, name="v16")
                g16 = vgp.tile([128, KT, 256], F16, name="g16")
                nc.vector.memset(v16[:, :, 256:257], 1.0)

                yacc = {}
                qrots = {}

                def transpose_tile(t):
                    qrot = qrots.pop(t)
                    tq = tp_ps.tile([128, 4, 128], F16, name="tq")
                    for h in range(4):
                        nc.tensor.transpose(tq[:, h, :], qrot[:, h, :],
                                            id_sb[:])
                    nc.vector.tensor_copy(qkT[:, t, :, :], tq[:])

                def phase1_tile(t):
                    ts = slice(t * 128, (t + 1) * 128)
                    xt2 = get_xchunk(b, t // 4)
                    if t % 4 == 0 and t // 4 + 1 < 4:
                        get_xchunk(b, t // 4 + 1)   # prefetch next stripe
                    tl = slice((t % 4) * 128, (t % 4) * 128 + 128)
                    qk_ps = mm_ps.tile([128, 512], F32, name="qk_ps")
                    vg_ps = mm_ps.tile([128, 512], F32, name="vg_ps")
                    for half, ps in ((0, qk_ps), (1, vg_ps)):
                        cs = slice(half * 512, half * 512 + 512)
                        # 3-term split as k-paired DoubleRows, regular strides:
                        # (xh@Wh + xl@Wh + xh@Wl), lo*lo dropped
                        for gi, (xp, wp) in enumerate(((0, 0), (1, 0), (0, 1))):
                            for k2 in range(KT // 2):
                                k = 2 * k2
                                nc.tensor.matmul(
                                    ps[:], xt2[:, xp, k:k + 2, tl],
                                    w2[:, wp, k:k + 2, cs],
                                    start=(gi == 0 and k2 == 0),
                                    stop=(gi == 2 and k2 == KT // 2 - 1),
                                    perf_mode=DR)
                    # qk -> f16 (ACT), stats + rope (DVE), xbar transpose (SP)
                    qk16 = p1w.tile([128, 4, 128], F16, name="qk16")
                    nc.scalar.copy(qk16[:], qk_ps[:].rearrange("p (h d) -> p h d", h=4))
                    ss = p1s.tile([128, 4], F32, name="ss")
                    sq = p1w.tile([128, 4, 128], F16, name="sq", bufs=2)
                    nc.vector.tensor_mul(sq[:], qk16[:], qk16[:])
                    nc.vector.tensor_reduce(ss[:], sq[:],
                                            axis=mybir.AxisListType.X,
                                            op=ALU.add)
                    rstd = _rsqrt_dve(nc, p1s, ss[:], 4, 1.0 / HD, 1, "rq")
                    h1, h2 = qk16[:, :, 0:64], qk16[:, :, 64:128]
                    cos_b = _bcast_mid(cos_sb[:, t, :], 4)
                    sin_b = _bcast_mid(sin_sb[:, t, :], 4)
                    ra = p1w.tile([128, 4, 64], F16, name="ra")
                    rb = p1w.tile([128, 4, 64], F16, name="rb")
                    rot = p1w.tile([128, 4, 128], F16, name="rot")
                    nc.vector.tensor_mul(ra[:], h1, cos_b)
                    nc.vector.tensor_mul(rb[:], h2, sin_b)
                    nc.vector.tensor_add(rot[:, :, 0:64], ra[:], rb[:])
                    nc.vector.tensor_mul(ra[:], h2, cos_b)
                    nc.vector.tensor_mul(rb[:], h1, sin_b)
                    nc.vector.tensor_sub(rot[:, :, 64:128], ra[:], rb[:])
                    qrot = p1w.tile([128, 4, 128], F16, name="qrot", bufs=2)
                    for h in range(4):
                        nc.vector.tensor_scalar_mul(qrot[:, h, :], in0=rot[:, h, :],
                                                    scalar1=rstd[:, h:h + 1])
                    qrots[t] = qrot
                    # v and gate
                    nc.scalar.copy(v16[:, t, 0:256], vg_ps[:, 0:256])
                    th = p1w.tile([128, 256], F16, name="th")
                    nc.scalar.activation(th[:], vg_ps[:, 256:512], AF.Tanh,
                                         scale=0.5 / WS)
                    nc.vector.scalar_tensor_tensor(
                        g16[:, t, :], th[:], 1.0, vg_ps[:, 256:512],
                        op0=ALU.add, op1=ALU.mult)

                def phase2_chunk(c):
                    # var-sequential; var0 sums/values extracted to SBUF right
                    # after its accumulators stop, so yacc rotates in 3 bufs
                    ex = {}
                    for var in range(2):
                        for m in range(2):
                            yacc[(var, m)] = yac_ps.tile([128, 512], F32,
                                                         name="yacc")
                        for jp in range(c):          # full off-diag pairs
                            j0 = 2 * jp
                            scp = sc_ps.tile([128, 512], F32, name="scp")
                            for jj in range(2):
                                nc.tensor.matmul(
                                    scp[:, jj * CH:(jj + 1) * CH],
                                    qkT[:, j0 + jj, 2 + var, :],
                                    qkT[:, 2 * c:2 * c + 2, var, :],
                                    start=True, stop=True)
                            probs = p2w.tile([128, 512], F16, name="probs",
                                             bufs=2)
                            nc.scalar.activation(probs[:], scp[:], AF.Exp,
                                                 scale=SCALE, bias=ebias[:])
                            for m in range(2):
                                for jj in range(2):
                                    nc.tensor.matmul(
                                        yacc[(var, m)][:, 0:VW],
                                        probs[:, jj * CH + m * 128:
                                              jj * CH + m * 128 + 128],
                                        v16[:, j0 + jj, :],
                                        start=(jp == 0 and jj == 0), stop=False)
                        # diagonal pair: jj1 only exists for the upper query
                        # half; upper-triangle blocks are masked
                        j0 = 2 * c
                        scp = sc_ps.tile([128, 512], F32, name="scp")
                        nc.tensor.matmul(scp[:, 0:CH], qkT[:, j0, 2 + var, :],
                                         qkT[:, 2 * c:2 * c + 2, var, :],
                                         start=True, stop=True)
                        nc.tensor.matmul(scp[:, CH:CH + 128],
                                         qkT[:, j0 + 1, 2 + var, :],
                                         qkT[:, 2 * c + 1, var, :],
                                         start=True, stop=True)
                        msc = p2w.tile([128, 384], F32, name="msc", bufs=1)
                        nc.vector.scalar_tensor_tensor(
                            msc[:], scp[:, 0:384], 1.0, mask_sb[:],
                            op0=ALU.mult, op1=ALU.add)
                        probs = p2w.tile([128, 512], F16, name="probs", bufs=2)
                        nc.scalar.activation(probs[:, 0:384], msc[:], AF.Exp,
                                             scale=SCALE, bias=ebias[:])
                        nc.tensor.matmul(yacc[(var, 0)][:, 0:VW],
                                         probs[:, 0:128], v16[:, j0, :],
                                         start=(c == 0), stop=True)
                        nc.tensor.matmul(yacc[(var, 1)][:, 0:VW],
                                         probs[:, 128:256], v16[:, j0, :],
                                         start=(c == 0), stop=False)
                        nc.tensor.matmul(yacc[(var, 1)][:, 0:VW],
                                         probs[:, 256:384], v16[:, j0 + 1, :],
                                         start=False, stop=True)
                        if var == 0:
                            for m in range(2):
                                t1 = p2s.tile([128, 256], F16, name="t1",
                                              bufs=2)
                                nc.scalar.copy(t1[:], yacc[(0, m)][:, 0:256])
                                s1c = p2s.tile([128, 1], F32, name="s1c",
                                               bufs=2)
                                nc.vector.tensor_copy(s1c[:],
                                                      yacc[(0, m)][:, 256:257])
                                ex[m] = (t1, s1c)
                    # ---- epilogue ----
                    ssy = p2s.tile([128, 2], F32, name="ssy")
                    sqy = p2s.tile([128, 2, 256], F32, name="sqy", bufs=1)
                    ygs = []
                    for m in range(2):
                        t1, s1c = ex[m]
                        y2a = yacc[(1, m)]
                        s2l = p2s.tile([128, 1], F32, name="s2l")
                        nc.vector.tensor_mul(s2l[:], y2a[:, 256:257], lam_sb[:])
                        rec = p2s.tile([128, 1], F32, name="rec")
                        nc.vector.reciprocal(rec[:], s2l[:])
                        rho = p2s.tile([128, 1], F32, name="rho")
                        nc.vector.tensor_scalar(out=rho[:], in0=rec[:],
                                                scalar1=s1c[:],
                                                scalar2=None, op0=ALU.mult)
                        # yt = rho*y2 - y1 (negated diff; sign fixed at out)
                        yt = p2s.tile([128, 256], F32, name="yt", bufs=1)
                        nc.vector.scalar_tensor_tensor(
                            yt[:], y2a[:, 0:256], rho[:], t1[:],
                            op0=ALU.mult, op1=ALU.subtract)
                        yg = p2s.tile([128, 256], F32, name="yg", bufs=2)
                        nc.gpsimd.tensor_mul(yg[:], yt[:], g16[:, 2 * c + m, :])
                        nc.vector.tensor_mul(sqy[:, m, :], yg[:], yg[:])
                        ygs.append(yg)
                    nc.vector.tensor_reduce(ssy[:], sqy[:],
                                            axis=mybir.AxisListType.X,
                                            op=ALU.add)
                    rsy = _rsqrt_dve(nc, p2s, ssy[:], 2, 1.0 / 256, 1, "ry")
                    for m in range(2):
                        qt = 2 * c + m
                        out_t = outp.tile([128, 256], F32, name="out_t")
                        nc.gpsimd.tensor_scalar(
                            out=out_t[:], in0=ygs[m][:],
                            scalar1=rsy[:, m:m + 1], scalar2=-ONE_MINUS_LI,
                            op0=ALU.mult, op1=ALU.mult)
                        nc.sync.dma_start(
                            y_d[b, qt * 128:(qt + 1) * 128, :], out_t[:])

                for step in range(NCH + 1):
                    if step < NCH:
                        phase1_tile(2 * step)
                        if step > 0:
                            transpose_tile(2 * step - 1)
                        phase1_tile(2 * step + 1)
                        transpose_tile(2 * step)
                    if step >= 1:
                        phase2_chunk(step - 1)
                    if step == NCH - 1:
                        transpose_tile(2 * step + 1)
    nc.compile()
    return nc


_NC = None


def prep_in_maps(hidden_states, W_qkv, lambda_q1, lambda_k1, lambda_q2,
                 lambda_k2, W_g):
    E4 = ml_dtypes.float8_e4m3
    x = np.asarray(hidden_states, dtype=np.float32)
    xt = np.ascontiguousarray(x.transpose(0, 2, 1))
    xh = xt.astype(E4)
    xl = (xt - xh.astype(np.float32)).astype(E4)
    W_qkv = np.asarray(W_qkv, dtype=np.float32)
    W_g = np.asarray(W_g, dtype=np.float32)

    t_ar = np.arange(T, dtype=np.float32)
    inv_freq = (1.0 / 10000.0 ** (np.arange(0, HD, 2, dtype=np.float32) / HD)
                ).astype(np.float32)
    freqs = np.outer(t_ar, inv_freq)
    cos = np.cos(freqs).astype(np.float16)
    sin = np.sin(freqs).astype(np.float16)

    masks = np.empty((128, 384), dtype=np.float32)
    kk = np.arange(128)[:, None]
    qq = np.arange(CH)[None, :]
    masks[:, 0:256] = np.where(kk <= qq, 0.0, NEG)
    masks[:, 256:384] = np.where(128 + kk <= qq[:, 128:256], 0.0, NEG)

    ident = np.eye(128, dtype=np.float16)
    lam1 = np.exp(np.sum(np.asarray(lambda_q1, np.float32)
                         * np.asarray(lambda_k1, np.float32), axis=-1))
    lam2 = np.exp(np.sum(np.asarray(lambda_q2, np.float32)
                         * np.asarray(lambda_k2, np.float32), axis=-1))
    lam = (lam1 - lam2 + LAMBDA_INIT).astype(np.float32)   # [8]

    in_maps = []
    for c in range(N_CORES):
        base = 2 * c * 384
        w_cols = [
            W_qkv[:, base:base + 128],            # q1
            W_qkv[:, base + 384:base + 512],      # q2
            W_qkv[:, base + 128:base + 256],      # k1
            W_qkv[:, base + 512:base + 640],      # k2
            W_qkv[:, base + 256:base + 384],      # v1
            W_qkv[:, base + 640:base + 768],      # v2
            W_g[:, c * 256:(c + 1) * 256],        # gate
        ]
        wcat = np.concatenate(w_cols, axis=1) * WS
        whp = wcat.astype(E4)
        wlp = (wcat - whp.astype(np.float32)).astype(E4)
        lamv = np.full((128, 1), 1.0 / lam[c], dtype=np.float32)
        in_maps.append({
            "xh": xh, "xl": xl, "wh": np.ascontiguousarray(whp),
            "wl": np.ascontiguousarray(wlp), "cos": cos, "sin": sin,
            "masks": masks, "lam": lamv, "ident": ident,
        })
    return in_maps


def kernel(hidden_states, W_qkv, lambda_q1, lambda_k1, lambda_q2, lambda_k2,
           W_g, **run_kwargs):
    global _NC
    if _NC is None:
        _NC = build()
    in_maps = prep_in_maps(hidden_states, W_qkv, lambda_q1, lambda_k1,
                           lambda_q2, lambda_k2, W_g)
    res = run_bass_kernel_spmd(_NC, in_maps, core_ids=list(range(N_CORES)),
                               **run_kwargs)
    out = np.empty((B, T, D), dtype=np.float32)
    for c in range(N_CORES):
        out[:, :, c * 256:(c + 1) * 256] = res.results[c]["y"]
    if run_kwargs:
        return out, res
    return out


# revision 3
# speedup vs baseline: 1.0201x; 1.0103x over previous
"""MixerDiffAttention Trainium2 kernel, v3 (unit-woven pipeline).

Sharding: 8 cores = 8 head-pairs (tensor parallel); each core does both
batches for its head pair (weight slice: 768 qkv + 256 gate cols).

Phase 1 (per 128-token tile):
  - qkv+gate GEMM as a 3-term split-fp8 DoubleRow product:
      x = xh + xl, W = Wh + Wl (fp8 e4m3, host-split);
      out = xh@Wh + xl@Wh + xh@Wl  (lo*lo dropped, ~0.1% error),
    48 DoubleRow matmuls per tile (k-paired, regular strides).
  - qk -> fp16 copy (ACT); sumsq via DVE square+reduce; Quake rsqrt;
    RoPE fp16 on DVE (2x); feature-major transpose on PE (fp16);
    v, gate(tanh-silu) -> fp16.
Phase 2 (per 256-query chunk, vars = even/odd heads):
  - transposed scores fp16 -> PSUM f32; probs = Exp(scale*s - 2) -> fp16
    (bias washes in softmax norm); AV fp16 accumulates [v|1] so row sums
    are free; diagonal pair computes only the 3 live 128x(128/256) blocks.
  - epilogue: rho = lam*s1/s2 via one reciprocal; yt = rho*y2 - y1;
    gate multiply; group RMS via square+reduce + Quake. All row scales
    (silu 2x, WS, s1, sign) wash in the group RMSNorm.

Scheduling: chunk c is emitted one tile-step behind, its units WOVEN
between the GEMM matmul groups of tiles 2c+2/2c+3 (and across the batch
boundary), so PE always has GEMM work while ACT exps cook. All engines
stream concurrently; PE is the bottleneck at ~85%+ occupancy.
"""
import sys
sys.path.insert(0, "/opt/trn_rl_repo")
import numpy as np
import ml_dtypes
import concourse.bass as bass
from concourse import bacc
import concourse.tile as tile
from concourse import mybir
from concourse.bass_utils import run_bass_kernel_spmd

F32 = mybir.dt.float32
F16 = mybir.dt.float16
F8 = mybir.dt.float8e4
AF = mybir.ActivationFunctionType
ALU = mybir.AluOpType
DR = mybir.MatmulPerfMode.DoubleRow

B, T, D, HD = 2, 2048, 2048, 128
KT = D // 128          # 16 contraction tiles of 128
TT = T // 128          # 16 token tiles
CH = 256               # query-chunk width
NCH = T // CH          # 8 chunks
VW = 257               # v width: 256 + ones column
TQ = T // 4            # x stripe chunk
N_CORES = 8
WS = 32.0              # host weight prescale (washes in rmsnorms)
LAMBDA_INIT = 0.8 - 0.6 * float(np.exp(-0.3 * 6))
ONE_MINUS_LI = 1.0 - LAMBDA_INIT
SCALE = float(HD ** -0.5)
EXP_BIAS = -2.0        # washes in softmax normalization; keeps f16 range
EPS = 1e-6
NEG = -1e9


def _bcast_mid(ap, n):
    # [P, F] AP -> [P, n, F] with a zero-stride middle dim
    return bass.AP(tensor=ap.tensor, offset=ap.offset,
                   ap=[ap.ap[0], [0, n], *ap.ap[1:]])


def _rsqrt_dve(nc, pool, ss_ap, width, scale1, iters, tag):
    """rstd = (ss*scale1 + EPS) ** -0.5 on DVE (Quake seed + Newton)."""
    I32 = mybir.dt.int32
    ms = pool.tile([128, width], F32, name=tag + "_ms")
    nc.vector.tensor_scalar(out=ms[:], in0=ss_ap, scalar1=scale1,
                            scalar2=EPS, op0=ALU.mult, op1=ALU.add)
    iv = pool.tile([128, width], I32, name=tag + "_iv")
    nc.vector.tensor_scalar(out=iv[:], in0=ms[:].bitcast(I32), scalar1=1,
                            scalar2=None, op0=ALU.logical_shift_right)
    y = pool.tile([128, width], F32, name=tag + "_y")
    nc.vector.tensor_scalar(out=y[:].bitcast(I32), in0=iv[:], scalar1=-1,
                            scalar2=0x5F3759DF, op0=ALU.mult, op1=ALU.add)
    a = pool.tile([128, width], F32, name=tag + "_a")
    u = pool.tile([128, width], F32, name=tag + "_u")
    for _ in range(iters):
        nc.vector.tensor_mul(a[:], y[:], y[:])
        nc.vector.tensor_mul(a[:], a[:], ms[:])
        nc.vector.tensor_scalar(out=u[:], in0=a[:], scalar1=-0.5, scalar2=1.5,
                                op0=ALU.mult, op1=ALU.add)
        nc.vector.tensor_mul(y[:], y[:], u[:])
    return y


def build(nb=B):
    nc = bacc.Bacc("TRN2", target_bir_lowering=False, debug=False,
                   num_devices=N_CORES)
    xh_d = nc.dram_tensor("xh", [nb, D, T], F8, kind="ExternalInput").ap()
    xl_d = nc.dram_tensor("xl", [nb, D, T], F8, kind="ExternalInput").ap()
    wh_d = nc.dram_tensor("wh", [D, 1024], F8, kind="ExternalInput").ap()
    wl_d = nc.dram_tensor("wl", [D, 1024], F8, kind="ExternalInput").ap()
    cos_d = nc.dram_tensor("cos", [T, 64], F16, kind="ExternalInput").ap()
    sin_d = nc.dram_tensor("sin", [T, 64], F16, kind="ExternalInput").ap()
    mask_d = nc.dram_tensor("masks", [128, 384], F32, kind="ExternalInput").ap()
    lam_d = nc.dram_tensor("lam", [128, 1], F32, kind="ExternalInput").ap()
    id_d = nc.dram_tensor("ident", [128, 128], F16, kind="ExternalInput").ap()
    y_d = nc.dram_tensor("y", [nb, T, 256], F32, kind="ExternalOutput").ap()

    with tile.TileContext(nc) as tc:
        with tc.tile_pool(name="consts", bufs=1) as consts, \
             tc.tile_pool(name="xtp", bufs=5) as xtp, \
             tc.tile_pool(name="qktp", bufs=2) as qktp, \
             tc.tile_pool(name="vgp", bufs=2) as vgp, \
             tc.tile_pool(name="p1w", bufs=2) as p1w, \
             tc.tile_pool(name="p1s", bufs=8) as p1s, \
             tc.tile_pool(name="p2w", bufs=3) as p2w, \
             tc.tile_pool(name="p2s", bufs=4) as p2s, \
             tc.tile_pool(name="outp", bufs=2) as outp, \
             tc.tile_pool(name="mm_ps", bufs=1, space="PSUM") as mm_ps, \
             tc.tile_pool(name="sc_ps", bufs=2, space="PSUM") as sc_ps, \
             tc.tile_pool(name="yac_ps", bufs=3, space="PSUM") as yac_ps, \
             tc.tile_pool(name="tp_ps", bufs=1, space="PSUM") as tp_ps:

            # ---- constants / weights (k-halved for a fast first tile) ----
            w2 = consts.tile([128, 2, KT, 1024], F8)
            wh_v = wh_d.rearrange("(k p) c -> p k c", p=128)
            wl_v = wl_d.rearrange("(k p) c -> p k c", p=128)
            nc.sync.dma_start(w2[:, 0, 0:8, :], wh_v[:, 0:8, :])
            nc.sync.dma_start(w2[:, 0, 8:16, :], wh_v[:, 8:16, :])
            xq_tiles = {}

            def get_xchunk(b, ch):
                if (b, ch) not in xq_tiles:
                    xq = xtp.tile([128, 2, KT, TQ], F8, name="xq")
                    bsl = slice(ch * TQ, (ch + 1) * TQ)
                    xhv = xh_d[b].rearrange("(k p) t -> p k t", p=128)
                    xlv = xl_d[b].rearrange("(k p) t -> p k t", p=128)
                    nc.sync.dma_start(xq[:, 0, :, :], xhv[:, :, bsl])
                    nc.sync.dma_start(xq[:, 1, :, :], xlv[:, :, bsl])
                    xq_tiles[(b, ch)] = xq
                return xq_tiles[(b, ch)]

            get_xchunk(0, 0)
            nc.sync.dma_start(w2[:, 1, 0:8, :], wl_v[:, 0:8, :])
            nc.sync.dma_start(w2[:, 1, 8:16, :], wl_v[:, 8:16, :])
            cos_sb = consts.tile([128, TT, 64], F16)
            nc.sync.dma_start(cos_sb[:], cos_d.rearrange("(t p) f -> p t f", p=128))
            sin_sb = consts.tile([128, TT, 64], F16)
            nc.sync.dma_start(sin_sb[:], sin_d.rearrange("(t p) f -> p t f", p=128))
            mask_sb = consts.tile([128, 384], F32)
            nc.sync.dma_start(mask_sb[:], mask_d)
            lam_sb = consts.tile([128, 1], F32)   # holds 1/lambda
            nc.sync.dma_start(lam_sb[:], lam_d)
            ebias = consts.tile([128, 1], F32)    # exp bias (washes in norm)
            nc.vector.memset(ebias[:], EXP_BIAS)
            id_sb = consts.tile([128, 128], F16)
            nc.sync.dma_start(id_sb[:], id_d)

            # ---- per-batch resources ----
            res = {}

            def batch_res(b):
                if b not in res:
                    qkT = qktp.tile([128, TT, 4, 128], F16, name="qkT")
                    v16 = vgp.tile([128, KT, VW], F16, name="v16")
                    g16 = vgp.tile([128, KT, 256], F16, name="g16")
                    nc.vector.memset(v16[:, :, 256:257], 1.0)
                    res[b] = (qkT, v16, g16, {})
                return res[b]

            qrots = {}

            def transpose_tile(b, t):
                qkT = batch_res(b)[0]
                qrot = qrots.pop((b, t))
                tq = tp_ps.tile([128, 4, 128], F16, name="tq")
                for h in range(4):
                    nc.tensor.transpose(tq[:, h, :], qrot[:, h, :], id_sb[:])
                nc.vector.tensor_copy(qkT[:, t, :, :], tq[:])

            def gemm_units(b, t):
                """6 closures of 8 DoubleRow matmuls + fused drains."""
                qkT, v16, g16, _ = batch_res(b)
                state = {}

                def grp(half, gi, xp, wp):
                    def emit():
                        if "x" not in state:
                            state["x"] = get_xchunk(b, t // 4)
                            if t % 4 == 0 and t // 4 + 1 < 4:
                                get_xchunk(b, t // 4 + 1)
                            state["qk"] = mm_ps.tile([128, 512], F32,
                                                     name="qk_ps")
                            state["vg"] = mm_ps.tile([128, 512], F32,
                                                     name="vg_ps")
                        xt2 = state["x"]
                        ps = state["qk"] if half == 0 else state["vg"]
                        tl = slice((t % 4) * 128, (t % 4) * 128 + 128)
                        cs = slice(half * 512, half * 512 + 512)
                        for k2 in range(KT // 2):
                            k = 2 * k2
                            nc.tensor.matmul(
                                ps[:], xt2[:, xp, k:k + 2, tl],
                                w2[:, wp, k:k + 2, cs],
                                start=(gi == 0 and k2 == 0),
                                stop=(gi == 2 and k2 == KT // 2 - 1),
                                perf_mode=DR)
                        if half == 0 and gi == 2:
                            qk_drain()
                        if half == 1 and gi == 2:
                            vg_drain()
                    return emit

                def qk_drain():
                    qk_ps = state["qk"]
                    qk16 = p1w.tile([128, 4, 128], F16, name="qk16")
                    nc.scalar.copy(qk16[:],
                                   qk_ps[:].rearrange("p (h d) -> p h d", h=4))
                    ss = p1s.tile([128, 4], F32, name="ss")
                    sq = p1w.tile([128, 4, 128], F16, name="sq", bufs=2)
                    nc.vector.tensor_mul(sq[:], qk16[:], qk16[:])
                    nc.vector.tensor_reduce(ss[:], sq[:],
                                            axis=mybir.AxisListType.X,
                                            op=ALU.add)
                    rstd = _rsqrt_dve(nc, p1s, ss[:], 4, 1.0 / HD, 1, "rq")
                    h1, h2 = qk16[:, :, 0:64], qk16[:, :, 64:128]
                    cos_b = _bcast_mid(cos_sb[:, t, :], 4)
                    sin_b = _bcast_mid(sin_sb[:, t, :], 4)
                    ra = p1w.tile([128, 4, 64], F16, name="ra")
                    rb = p1w.tile([128, 4, 64], F16, name="rb")
                    rot = p1w.tile([128, 4, 128], F16, name="rot")
                    nc.vector.tensor_mul(ra[:], h1, cos_b)
                    nc.vector.tensor_mul(rb[:], h2, sin_b)
                    nc.vector.tensor_add(rot[:, :, 0:64], ra[:], rb[:])
                    nc.vector.tensor_mul(ra[:], h2, cos_b)
                    nc.vector.tensor_mul(rb[:], h1, sin_b)
                    nc.vector.tensor_sub(rot[:, :, 64:128], ra[:], rb[:])
                    qrot = p1w.tile([128, 4, 128], F16, name="qrot", bufs=2)
                    for h in range(4):
                        nc.vector.tensor_scalar_mul(qrot[:, h, :],
                                                    in0=rot[:, h, :],
                                                    scalar1=rstd[:, h:h + 1])
                    qrots[(b, t)] = qrot

                def vg_drain():
                    vg_ps = state["vg"]
                    nc.scalar.copy(v16[:, t, 0:256], vg_ps[:, 0:256])
                    th = p1w.tile([128, 256], F16, name="th")
                    nc.scalar.activation(th[:], vg_ps[:, 256:512], AF.Tanh,
                                         scale=0.5 / WS)
                    nc.vector.scalar_tensor_tensor(
                        g16[:, t, :], th[:], 1.0, vg_ps[:, 256:512],
                        op0=ALU.add, op1=ALU.mult)

                return [grp(0, 0, 0, 0), grp(0, 1, 1, 0), grp(0, 2, 0, 1),
                        grp(1, 0, 0, 0), grp(1, 1, 1, 0), grp(1, 2, 0, 1)]

            def chunk_units(b, c, split_exp=False):
                """Off-diag jp units, diag units, epilogue closures."""
                qkT, v16, g16, yacc = batch_res(b)
                ex = {}

                def offd(var, jp):
                    st = {}

                    def emit_sc():
                        j0 = 2 * jp
                        scp = sc_ps.tile([128, 512], F32, name="scp")
                        for jj in range(2):
                            nc.tensor.matmul(
                                scp[:, jj * CH:(jj + 1) * CH],
                                qkT[:, j0 + jj, 2 + var, :],
                                qkT[:, 2 * c:2 * c + 2, var, :],
                                start=True, stop=True)
                        probs = p2w.tile([128, 512], F16, name="probs", bufs=2)
                        if split_exp:
                            nc.scalar.activation(probs[:, 0:256],
                                                 scp[:, 0:256], AF.Exp,
                                                 scale=SCALE, bias=ebias[:])
                            nc.scalar.activation(probs[:, 256:512],
                                                 scp[:, 256:512], AF.Exp,
                                                 scale=SCALE, bias=ebias[:])
                        else:
                            nc.scalar.activation(probs[:], scp[:], AF.Exp,
                                                 scale=SCALE, bias=ebias[:])
                        st["p"] = probs

                    def emit_av():
                        if (var, 0) not in yacc:
                            for m in range(2):
                                yacc[(var, m)] = yac_ps.tile(
                                    [128, 512], F32, name="yacc")
                        j0 = 2 * jp
                        probs = st["p"]
                        for m in range(2):
                            for jj in range(2):
                                nc.tensor.matmul(
                                    yacc[(var, m)][:, 0:VW],
                                    probs[:, jj * CH + m * 128:
                                          jj * CH + m * 128 + 128],
                                    v16[:, j0 + jj, :],
                                    start=(jp == 0 and jj == 0), stop=False)
                    return emit_sc, emit_av

                def diag(var):
                    st = {}

                    def emit_sc():
                        j0 = 2 * c
                        scp = sc_ps.tile([128, 512], F32, name="scp")
                        nc.tensor.matmul(scp[:, 0:CH], qkT[:, j0, 2 + var, :],
                                         qkT[:, 2 * c:2 * c + 2, var, :],
                                         start=True, stop=True)
                        nc.tensor.matmul(scp[:, CH:CH + 128],
                                         qkT[:, j0 + 1, 2 + var, :],
                                         qkT[:, 2 * c + 1, var, :],
                                         start=True, stop=True)
                        msc = p2w.tile([128, 384], F32, name="msc", bufs=1)
                        nc.vector.scalar_tensor_tensor(
                            msc[:], scp[:, 0:384], 1.0, mask_sb[:],
                            op0=ALU.mult, op1=ALU.add)
                        probs = p2w.tile([128, 512], F16, name="probs", bufs=2)
                        nc.scalar.activation(probs[:, 0:384], msc[:], AF.Exp,
                                             scale=SCALE, bias=ebias[:])
                        st["p"] = probs

                    def emit_av():
                        if (var, 0) not in yacc:   # c == 0 path
                            for m in range(2):
                                yacc[(var, m)] = yac_ps.tile(
                                    [128, 512], F32, name="yacc")
                        j0 = 2 * c
                        probs = st["p"]
                        nc.tensor.matmul(yacc[(var, 0)][:, 0:VW],
                                         probs[:, 0:128], v16[:, j0, :],
                                         start=(c == 0), stop=True)
                        nc.tensor.matmul(yacc[(var, 1)][:, 0:VW],
                                         probs[:, 128:256], v16[:, j0, :],
                                         start=(c == 0), stop=False)
                        nc.tensor.matmul(yacc[(var, 1)][:, 0:VW],
                                         probs[:, 256:384], v16[:, j0 + 1, :],
                                         start=False, stop=True)
                        if var == 0:
                            for m in range(2):
                                t1 = p2s.tile([128, 256], F16, name="t1",
                                              bufs=2)
                                nc.scalar.copy(t1[:], yacc[(0, m)][:, 0:256])
                                s1c = p2s.tile([128, 1], F32, name="s1c",
                                               bufs=2)
                                nc.vector.tensor_copy(
                                    s1c[:], yacc[(0, m)][:, 256:257])
                                ex[m] = (t1, s1c)
                            yacc.pop((0, 0))
                            yacc.pop((0, 1))
                    return emit_sc, emit_av

                def epilogue():
                    ssy = p2s.tile([128, 2], F32, name="ssy")
                    sqy = p2s.tile([128, 2, 256], F32, name="sqy", bufs=1)
                    ygs = []
                    for m in range(2):
                        t1, s1c = ex[m]
                        y2a = yacc.pop((1, m))
                        s2l = p2s.tile([128, 1], F32, name="s2l")
                        nc.vector.tensor_mul(s2l[:], y2a[:, 256:257],
                                             lam_sb[:])
                        rec = p2s.tile([128, 1], F32, name="rec")
                        nc.vector.reciprocal(rec[:], s2l[:])
                        rho = p2s.tile([128, 1], F32, name="rho")
                        nc.vector.tensor_scalar(out=rho[:], in0=rec[:],
                                                scalar1=s1c[:],
                                                scalar2=None, op0=ALU.mult)
                        # yt = rho*y2 - y1 (negated; sign folded into out)
                        yt = p2s.tile([128, 256], F32, name="yt", bufs=1)
                        nc.vector.scalar_tensor_tensor(
                            yt[:], y2a[:, 0:256], rho[:], t1[:],
                            op0=ALU.mult, op1=ALU.subtract)
                        yg = p2s.tile([128, 256], F32, name="yg", bufs=2)
                        nc.gpsimd.tensor_mul(yg[:], yt[:], g16[:, 2 * c + m, :])
                        nc.vector.tensor_mul(sqy[:, m, :], yg[:], yg[:])
                        ygs.append(yg)
                    nc.vector.tensor_reduce(ssy[:], sqy[:],
                                            axis=mybir.AxisListType.X,
                                            op=ALU.add)
                    rsy = _rsqrt_dve(nc, p2s, ssy[:], 2, 1.0 / 256, 1, "ry")
                    for m in range(2):
                        qt = 2 * c + m
                        out_t = outp.tile([128, 256], F32, name="out_t")
                        nc.gpsimd.tensor_scalar(
                            out=out_t[:], in0=ygs[m][:],
                            scalar1=rsy[:, m:m + 1], scalar2=-ONE_MINUS_LI,
                            op0=ALU.mult, op1=ALU.mult)
                        nc.sync.dma_start(
                            y_d[b, qt * 128:(qt + 1) * 128, :], out_t[:])

                def lag2(pairs):
                    # sc_k ... sc_{k+1} av_k ... : AV lags its scores by
                    # two slots so exps hide behind woven GEMM groups
                    out, pend = [], []
                    for sc, av in pairs:
                        out.append(sc)
                        pend.append(av)
                        if len(pend) > 1:
                            out.append(pend.pop(0))
                    out.extend(pend)
                    return out

                head = lag2([offd(0, jp) for jp in range(c)])
                tail = (lag2([diag(0)]) + lag2([offd(1, jp) for jp in range(c)])
                        + lag2([diag(1)]) + [epilogue])
                return head, tail

            def weave(gemms, works):
                # distribute works evenly across gemm groups
                g, w = list(gemms), list(works)
                if not g:
                    return w
                out = []
                per = len(w) / len(g)
                acc = 0.0
                for gi, gu in enumerate(g):
                    out.append(gu)
                    acc += per
                    while acc >= 1.0 and w:
                        out.append(w.pop(0))
                        acc -= 1.0
                out.extend(w)
                return out

            # ---- global schedule: tile-steps with chunks lagging by one ----
            steps = [(b, s) for b in range(nb) for s in range(NCH)]
            for i, (b, s) in enumerate(steps):
                ga = gemm_units(b, 2 * s)
                gb = gemm_units(b, 2 * s + 1)
                if i >= 1:
                    pb, ps_ = steps[i - 1]
                    head, tail = chunk_units(pb, ps_)
                else:
                    head, tail = [], []
                if i >= 1:
                    transpose_tile(pb, 2 * ps_ + 1)
                for u in weave(ga, head):
                    u()
                for u in weave(gb, tail):
                    u()
                transpose_tile(b, 2 * s)
            # final chunk (last batch) — no GEMM cover; split exps to
            # shorten the dependency ladder
            transpose_tile(nb - 1, TT - 1)
            head, tail = chunk_units(nb - 1, NCH - 1, split_exp=True)
            for u in head + tail:
                u()
    nc.compile()
    return nc


_NC = None


def prep_in_maps(hidden_states, W_qkv, lambda_q1, lambda_k1, lambda_q2,
                 lambda_k2, W_g):
    E4 = ml_dtypes.float8_e4m3
    x = np.asarray(hidden_states, dtype=np.float32)
    xt = np.ascontiguousarray(x.transpose(0, 2, 1))
    xh = xt.astype(E4)
    xl = (xt - xh.astype(np.float32)).astype(E4)
    W_qkv = np.asarray(W_qkv, dtype=np.float32)
    W_g = np.asarray(W_g, dtype=np.float32)

    t_ar = np.arange(T, dtype=np.float32)
    inv_freq = (1.0 / 10000.0 ** (np.arange(0, HD, 2, dtype=np.float32) / HD)
                ).astype(np.float32)
    freqs = np.outer(t_ar, inv_freq)
    cos = np.cos(freqs).astype(np.float16)
    sin = np.sin(freqs).astype(np.float16)

    masks = np.empty((128, 384), dtype=np.float32)
    kk = np.arange(128)[:, None]
    qq = np.arange(CH)[None, :]
    masks[:, 0:256] = np.where(kk <= qq, 0.0, NEG)
    masks[:, 256:384] = np.where(128 + kk <= qq[:, 128:256], 0.0, NEG)

    ident = np.eye(128, dtype=np.float16)
    lam1 = np.exp(np.sum(np.asarray(lambda_q1, np.float32)
                         * np.asarray(lambda_k1, np.float32), axis=-1))
    lam2 = np.exp(np.sum(np.asarray(lambda_q2, np.float32)
                         * np.asarray(lambda_k2, np.float32), axis=-1))
    lam = (lam1 - lam2 + LAMBDA_INIT).astype(np.float32)   # [8]

    in_maps = []
    for c in range(N_CORES):
        base = 2 * c * 384
        w_cols = [
            W_qkv[:, base:base + 128],            # q1
            W_qkv[:, base + 384:base + 512],      # q2
            W_qkv[:, base + 128:base + 256],      # k1
            W_qkv[:, base + 512:base + 640],      # k2
            W_qkv[:, base + 256:base + 384],      # v1
            W_qkv[:, base + 640:base + 768],      # v2
            W_g[:, c * 256:(c + 1) * 256],        # gate
        ]
        wcat = np.concatenate(w_cols, axis=1) * WS
        whp = wcat.astype(E4)
        wlp = (wcat - whp.astype(np.float32)).astype(E4)
        lamv = np.full((128, 1), 1.0 / lam[c], dtype=np.float32)
        in_maps.append({
            "xh": xh, "xl": xl, "wh": np.ascontiguousarray(whp),
            "wl": np.ascontiguousarray(wlp), "cos": cos, "sin": sin,
            "masks": masks, "lam": lamv, "ident": ident,
        })
    return in_maps


def kernel(hidden_states, W_qkv, lambda_q1, lambda_k1, lambda_q2, lambda_k2,
           W_g, **run_kwargs):
    global _NC
    if _NC is None:
        _NC = build()
    in_maps = prep_in_maps(hidden_states, W_qkv, lambda_q1, lambda_k1,
                           lambda_q2, lambda_k2, W_g)
    res = run_bass_kernel_spmd(_NC, in_maps, core_ids=list(range(N_CORES)),
                               **run_kwargs)
    out = np.empty((B, T, D), dtype=np.float32)
    for c in range(N_CORES):
        out[:, :, c * 256:(c + 1) * 256] = res.results[c]["y"]
    if run_kwargs:
        return out, res
    return out
